# revision 23
# baseline (speedup 1.0000x reference)
"""Trainium2 Bass kernel for BiDAF-style bidirectional attention.

Reference math (per batch b):
    sim[c,q]  = q[q]·wq + c[c]·wc + sum_e wm[e]*question[q,e]*context[c,e]
    c2q[c,:]  = softmax_q(sim[c,:]) @ question          # (C, E)
    q2c[:]    = softmax_c(max_q sim[c,:]) @ context     # (E,)
    out[c,:]  = [context | c2q | context*c2q | context*q2c]

Sharding: pure data parallel over batch (B=16 -> 2 batches per core x 8 cores).

Two-pass pipeline, one batch phase-shifted against the other:
  - all DMA loads are emitted up front; output cols 0:E are a verbatim
    copy of the context, so "copy-through" stores stream them out right
    after each group load lands -- DMA is busy from the first microsecond.
  - pass A (per pair of context tiles): PE transpose -> f32r sim matmul
    (wc folded in as an extra output column) -> qw add + rowmax -> exp
    (bf16 attention weights).
  - pass B (per pair): attention-weight transpose (4-deep ring packed in
    one PSUM bank) -> bf16 c2q matmul whose 257th column is the softmax
    row-sum (ones column appended to the question, padded to N=258) ->
    reciprocal + row rescale -> ctx*c2q on gpsimd -> store cols E:3E.
  - emission is a modulo software pipeline over the global pair index:
    pass B lags pass A by 5 pairs and is emitted first within each round,
    so every engine's in-order queue meets its operands already ready.
    The serial q2c epilogue chain is split into small pieces (pre / two
    matmul halves / fin) spread across rounds so it never parks mid-queue
    in front of ready pass-B work; ctx*q2c stores ride the scalar-engine
    HWDGE ring to dodge sync-ring head-of-line blocking.
  - q2c rank-1 matmuls use a bf16 shadow of the context cast during
    pass A; the tail is pure DMA (stores of the last batch's columns).
"""

import numpy as np

import concourse.bass as bass
import concourse.tile as tile
import concourse.mybir as mybir
from concourse import bacc
from concourse.bass_utils import run_bass_kernel_spmd
from concourse.masks import make_identity

B, C, Q, E = 16, 2048, 128, 256
NCORES = 8
BPC = B // NCORES          # batches per core
NT = C // 128              # context tiles per batch
NG = NT // 4               # groups of 4 tiles
NP = NT // 2               # pairs per batch
F32 = mybir.dt.float32
F32R = mybir.dt.float32r
BF16 = mybir.dt.bfloat16
AX = mybir.AxisListType.X
EXP = mybir.ActivationFunctionType.Exp
CPY = mybir.ActivationFunctionType.Copy


class _Ctx:
    pass


def _body(tc, out_ext, ctx_in, q_in, wq_in, wc_in, wm_in):
    nc = tc.nc
    with (
        tc.tile_pool(name="singles", bufs=1) as singles,
        tc.tile_pool(name="stgp", bufs=BPC * NG) as stgp,
        tc.tile_pool(name="qside", bufs=2) as qside,
        tc.tile_pool(name="xbfp", bufs=BPC * NG) as xbfp,
        tc.tile_pool(name="work", bufs=6) as work,
        tc.tile_pool(name="pers", bufs=2 * NP) as pers,
        tc.tile_pool(name="statsp", bufs=2) as statsp,
        tc.tile_pool(name="ps_xct", bufs=2, space="PSUM") as ps_xct,
        tc.tile_pool(name="ps_sim", bufs=2, space="PSUM") as ps_sim,
        tc.tile_pool(name="ps_pt", bufs=1, space="PSUM") as ps_pt,
        tc.tile_pool(name="ps_c2q", bufs=2, space="PSUM") as ps_c2q,
        tc.tile_pool(name="ps_misc", bufs=1, space="PSUM") as ps_misc,
    ):
        # ---- constants + params ------------------------------------------
        ident = singles.tile([128, 128], F32)
        make_identity(nc, ident)
        ident_bf = singles.tile([128, 128], BF16)
        make_identity(nc, ident_bf)
        ones_r = singles.tile([1, 128], F32)
        nc.vector.memset(ones_r, 1.0)
        ones_c = singles.tile([128, 1], F32)
        nc.vector.memset(ones_c, 1.0)
        wq_sb = singles.tile([128, 2], F32)
        nc.sync.dma_start(out=wq_sb, in_=wq_in.rearrange("(j p) -> p j", p=128))
        wc_sb = singles.tile([128, 2], F32)
        nc.sync.dma_start(out=wc_sb, in_=wc_in.rearrange("(j p) -> p j", p=128))
        wm_sb = singles.tile([128, 2], F32)
        nc.sync.dma_start(out=wm_sb, in_=wm_in.rearrange("(j p) -> p j", p=128))

        # ---- all loads up front + copy-through of cols 0:E ---------------
        bs = []
        for b in range(BPC):
            st = _Ctx()
            bs.append(st)
            st.qm = qside.tile([128, E], F32, tag="qm", name="qm")
            nc.sync.dma_start(out=st.qm, in_=q_in[b])
        for b in range(BPC):
            st = bs[b]
            st.stgs = []
            for g in range(NG):
                stg = stgp.tile([128, 4, 4 * E], F32, tag="stg", name="stg")
                st.stgs.append(stg)
                nc.sync.dma_start(
                    out=stg[:, :, 0:E],
                    in_=ctx_in[b, g * 512 : (g + 1) * 512, :].rearrange(
                        "(t p) e -> p t e", p=128
                    ),
                )
                if b == 0:
                    # out[:, :, 0:E] is exactly the context: stream it now
                    nc.sync.dma_start(
                        out=out_ext[b, g * 512 : (g + 1) * 512, 0:E].rearrange(
                            "(t p) f -> p t f", p=128
                        ),
                        in_=stg[:, :, 0:E],
                    )

        def copythru1(g):
            # batch 1's copy-through rides the gpsimd SWDGE queue mid-kernel:
            # always-ready filler for store-supply stalls on the sync queue
            nc.gpsimd.dma_start(
                out=out_ext[1, g * 512 : (g + 1) * 512, 0:E].rearrange(
                    "(t p) f -> p t f", p=128
                ),
                in_=bs[1].stgs[g][:, :, 0:E],
            )

        # ---- question-side prep for both batches -------------------------
        for b in range(BPC):
            st = bs[b]
            qm = st.qm
            qmt_ps = ps_xct.tile([128, E], F32, tag="xct", name="qmt_ps")
            for j in range(2):
                nc.tensor.transpose(
                    qmt_ps[:, j * 128 : (j + 1) * 128],
                    qm[:, j * 128 : (j + 1) * 128],
                    ident,
                )
            qmt_sb = qside.tile([128, E], F32, tag="qmt", name="qmt_sb")
            nc.vector.tensor_copy(out=qmt_sb, in_=qmt_ps)
            # question in bf16 with a ones column appended: the c2q matmul
            # then emits the softmax row-sum as its 257th output column
            # (padded to N=258 to keep the PE output width even).
            st.qm_bf = qside.tile([128, E + 2], BF16, tag="qmbf", name="qm_bf")
            nc.vector.tensor_copy(out=st.qm_bf[:, 0:E], in_=qm)
            nc.vector.memset(st.qm_bf[:, E : E + 1], 1.0)
            nc.vector.memset(st.qm_bf[:, E + 1 : E + 2], 0.0)
            # rhs_aug[:, j, 0:128] = wm-chunk * QmT-chunk ; [:, j, 128] = wc
            # cols 129:256 are zero pad so the fp32r matmul runs at N=256.
            st.rhs_aug = qside.tile([128, 2, E], F32R, tag="rhs_aug",
                                    name="rhs_aug")
            for j in range(2):
                nc.vector.tensor_scalar_mul(
                    st.rhs_aug[:, j, 0:128],
                    qmt_sb[:, j * 128 : (j + 1) * 128],
                    wm_sb[:, j : j + 1],
                )
                nc.vector.tensor_copy(
                    out=st.rhs_aug[:, j, 128:129], in_=wc_sb[:, j : j + 1]
                )
                nc.vector.tensor_scalar_mul(
                    st.rhs_aug[:, j, 129:256],
                    qmt_sb[:, j * 128 : (j + 1) * 128][:, 0:127],
                    0.0,
                )
            qw_ps = ps_misc.tile([1, 128], F32, tag="misc", name="qw_ps")
            for j in range(2):
                nc.tensor.matmul(
                    qw_ps,
                    wq_sb[:, j : j + 1],
                    qmt_sb[:, j * 128 : (j + 1) * 128],
                    start=(j == 0),
                    stop=(j == 1),
                )
            qw_row = qside.tile([1, 128], F32, tag="qw_row", name="qw_row")
            nc.vector.tensor_copy(out=qw_row, in_=qw_ps)
            qwb_ps = ps_misc.tile([128, 128], F32, tag="misc", name="qwb_ps")
            nc.tensor.matmul(qwb_ps, ones_r, qw_row, start=True, stop=True)
            st.qwb2 = qside.tile([128, 2, 128], F32, tag="qwb2", name="qwb2")
            nc.vector.tensor_copy(out=st.qwb2[:, 0, :], in_=qwb_ps)
            nc.vector.tensor_copy(out=st.qwb2[:, 1, :], in_=qwb_ps)
            st.mstat = statsp.tile([128, NT], F32, tag="mstat", name="mstat")
            st.p_sbs = {}
            st.recips = {}
            st.xcbfs = []

        # ---- pass A: sim + softmax stats + exp ---------------------------
        def pass_a(b, k):
            st = bs[b]
            g, h = k // 2, k % 2
            stg = st.stgs[g]
            if h == 0:
                xcbf = xbfp.tile([128, 4, E], BF16, tag="xcbf", name="xcbf")
                nc.scalar.copy(out=xcbf, in_=stg[:, :, 0:E])
                st.xcbfs.append(xcbf)
            xct_ps = ps_xct.tile([128, 2, E], F32, tag="xct", name="xct_ps")
            for i in range(2):
                for j in range(2):
                    nc.tensor.transpose(
                        xct_ps[:, i, j * 128 : (j + 1) * 128],
                        stg[:, 2 * h + i, j * 128 : (j + 1) * 128],
                        ident,
                    )
            xct_sb = work.tile([128, 2, E], F32R, tag="xct_sb", name="xct_sb")
            nc.vector.tensor_copy(out=xct_sb, in_=xct_ps)
            sim_ps = ps_sim.tile([128, 2, E], F32, tag="sim", name="sim_ps")
            for i in range(2):
                for j in range(2):
                    nc.tensor.matmul(
                        sim_ps[:, i, :],
                        xct_sb[:, i, j * 128 : (j + 1) * 128],
                        st.rhs_aug[:, j, :],
                        start=(j == 0),
                        stop=(j == 1),
                    )
            sim_in = work.tile([128, 2, 128], F32, tag="sim_in", name="sim_in")
            nc.vector.tensor_add(sim_in, sim_ps[:, :, 0:128], st.qwb2)
            neg_m = work.tile([128, 2], F32, tag="neg_m", name="neg_m")
            nc.vector.reduce_max(out=neg_m, in_=sim_in, axis=AX, negate=True)
            nc.vector.tensor_sub(
                st.mstat[:, 2 * k : 2 * k + 2], sim_ps[:, :, 128], neg_m
            )
            p_sb = pers.tile([128, 2, 128], BF16, tag="p_sb", name="p_sb")
            for i in range(2):
                nc.scalar.activation(
                    out=p_sb[:, i, :],
                    in_=sim_in[:, i, :],
                    func=EXP,
                    bias=neg_m[:, i : i + 1],
                    scale=1.0,
                )
            st.p_sbs[k] = p_sb

        # ---- pass B: c2q + ctx*c2q + store cols E:3E ---------------------
        def pass_b(b, k):
            st = bs[b]
            g, h = k // 2, k % 2
            stg = st.stgs[g]
            p_sb = st.p_sbs[k]
            pt_ps = st.pt_ring[:, k % 4, :, :]
            for i in range(2):
                nc.tensor.transpose(pt_ps[:, i, :], p_sb[:, i, :], ident_bf)
            pt_sb = work.tile([128, 2, 128], BF16, tag="pt_sb", name="pt_sb")
            nc.vector.tensor_copy(out=pt_sb, in_=pt_ps)
            recip = work.tile([128, 2], F32, tag="recip", name="recip")
            for i in range(2):
                c2q_ps = ps_c2q.tile(
                    [128, E + 2], F32, tag="c2q", name="c2q_ps"
                )
                nc.tensor.matmul(
                    c2q_ps, pt_sb[:, i, :], st.qm_bf, start=True, stop=True
                )
                nc.vector.reciprocal(
                    out=recip[:, i : i + 1], in_=c2q_ps[:, E : E + 1]
                )
                nc.scalar.activation(
                    out=stg[:, 2 * h + i, E : 2 * E],
                    in_=c2q_ps[:, 0:E],
                    func=CPY,
                    scale=recip[:, i : i + 1],
                )
            nc.gpsimd.tensor_mul(
                stg[:, 2 * h : 2 * h + 2, 2 * E : 3 * E],
                stg[:, 2 * h : 2 * h + 2, 0:E],
                stg[:, 2 * h : 2 * h + 2, E : 2 * E],
            )
            r0 = g * 512 + h * 256
            nc.sync.dma_start(
                out=out_ext[b, r0 : r0 + 256, E : 3 * E].rearrange(
                    "(t p) f -> p t f", p=128
                ),
                in_=stg[:, 2 * h : 2 * h + 2, E : 3 * E],
            )

        # ---- q2c epilogue: softmax over C, broadcast weights -------------
        def ep_pre(b):
            st = bs[b]
            mstat = st.mstat
            r1 = statsp.tile([128, 1], F32, tag="r1", name="r1")
            nc.vector.reduce_max(out=r1, in_=mstat, axis=AX)
            r1t_ps = ps_misc.tile([1, 128], F32, tag="misc", name="r1t_ps")
            nc.tensor.transpose(r1t_ps, r1, ident)
            neg_gmax = statsp.tile([1, 1], F32, tag="gmax", name="neg_gmax")
            nc.vector.reduce_max(
                out=neg_gmax, in_=r1t_ps, axis=AX, negate=True
            )
            ngb_ps = ps_misc.tile([128, 1], F32, tag="misc", name="ngb_ps")
            nc.tensor.matmul(ngb_ps, ones_r, neg_gmax, start=True, stop=True)
            ngb_sb = statsp.tile([128, 1], F32, tag="ngb", name="ngb_sb")
            nc.vector.tensor_copy(out=ngb_sb, in_=ngb_ps)
            st.e_sb = statsp.tile([128, NT], BF16, tag="e_sb", name="e_sb")
            s_col = statsp.tile([128, 1], F32, tag="s_col", name="s_col")
            nc.scalar.activation(
                out=st.e_sb, in_=mstat, func=EXP, bias=ngb_sb, scale=1.0,
                accum_out=s_col,
            )
            tot_ps = ps_misc.tile([1, 1], F32, tag="misc", name="tot_ps")
            nc.tensor.matmul(tot_ps, s_col, ones_c, start=True, stop=True)
            st.rt_sb = statsp.tile([1, 1], F32, tag="rt", name="rt_sb")
            nc.vector.reciprocal(out=st.rt_sb, in_=tot_ps)

        def ep_q2c(b, half):
            st = bs[b]
            if half == 0:
                st.q2c_ps = ps_misc.tile([1, E], F32, tag="misc",
                                         name="q2c_ps")
            for t in range(half * NT // 2, (half + 1) * NT // 2):
                nc.tensor.matmul(
                    st.q2c_ps,
                    st.e_sb[:, t : t + 1],
                    st.xcbfs[t // 4][:, t % 4, :],
                    start=(t == 0),
                    stop=(t == NT - 1),
                )

        def ep_fin(b):
            st = bs[b]
            q2c_sb = statsp.tile([1, E], F32, tag="q2c_sb", name="q2c_sb")
            nc.scalar.activation(
                out=q2c_sb, in_=st.q2c_ps, func=CPY, scale=st.rt_sb
            )
            q2cb_ps = ps_misc.tile([128, E], F32, tag="misc", name="q2cb_ps")
            nc.tensor.matmul(q2cb_ps, ones_r, q2c_sb, start=True, stop=True)
            st.q2cb2 = statsp.tile([128, 2, E], F32, tag="q2cb", name="q2cb2")
            nc.vector.tensor_copy(out=st.q2cb2[:, 0, :], in_=q2cb_ps)
            nc.vector.tensor_copy(out=st.q2cb2[:, 1, :], in_=q2cb_ps)

        # ---- ctx * q2c + store cols 3E:4E --------------------------------
        def stage3(b, g):
            st = bs[b]
            stg = st.stgs[g]
            for h in range(2):
                eng = nc.vector if h == 0 else nc.gpsimd
                eng.tensor_mul(
                    stg[:, 2 * h : 2 * h + 2, 3 * E : 4 * E],
                    stg[:, 2 * h : 2 * h + 2, 0:E],
                    st.q2cb2,
                )
            nc.scalar.dma_start(
                out=out_ext[
                    b, g * 512 : (g + 1) * 512, 3 * E : 4 * E
                ].rearrange("(t p) f -> p t f", p=128),
                in_=stg[:, :, 3 * E : 4 * E],
            )

        # ---- schedule ----------------------------------------------------
        for b in range(BPC):
            bs[b].pt_ring = ps_pt.tile(
                [128, 4, 2, 128], BF16, tag="pt", name="pt_ring"
            )
        # Modulo schedule over global pair index kk = b*NP + k.  Pass B lags
        # pass A by LAG pairs; pass B is emitted first inside each round
        # (its inputs are oldest, hence ready).  The serial q2c epilogue
        # chain is split into small pieces spread across rounds so it never
        # parks mid-queue in front of ready pass-B work.
        LAG = 5
        TOT = BPC * NP
        for r in range(TOT + LAG + NG):
            if r >= LAG and r - LAG < TOT:
                kk = r - LAG
                pass_b(kk // NP, kk % NP)
            if r < TOT:
                pass_a(r // NP, r % NP)
            if NP <= r < NP + 2 * NG and (r - NP) % 2 == 1:
                copythru1((r - NP) // 2)
            if r == NP:
                ep_pre(0)
            elif r == NP + 1:
                ep_q2c(0, 0)
            elif r == NP + 2:
                ep_q2c(0, 1)
            elif r == NP + 3:
                ep_fin(0)
            elif NP + 4 <= r < NP + 4 + 2 * NG and (r - NP) % 2 == 0:
                stage3(0, (r - (NP + 4)) // 2)
            if r == TOT:
                ep_pre(1)
            elif r == TOT + 1:
                ep_q2c(1, 0)
                ep_q2c(1, 1)
            elif r == TOT + 2:
                ep_fin(1)
            elif TOT + 3 <= r < TOT + 3 + NG:
                stage3(1, r - (TOT + 3))


_NC_CACHE = None


def _build():
    global _NC_CACHE
    if _NC_CACHE is not None:
        return _NC_CACHE
    nc = bacc.Bacc(
        "TRN2", target_bir_lowering=False, debug=False, num_devices=NCORES
    )
    ctx_in = nc.dram_tensor("context", [BPC, C, E], F32, kind="ExternalInput").ap()
    q_in = nc.dram_tensor("question", [BPC, Q, E], F32, kind="ExternalInput").ap()
    wq_in = nc.dram_tensor("w_question", [E], F32, kind="ExternalInput").ap()
    wc_in = nc.dram_tensor("w_context", [E], F32, kind="ExternalInput").ap()
    wm_in = nc.dram_tensor("w_multiple", [E], F32, kind="ExternalInput").ap()
    out_ext = nc.dram_tensor("out", [BPC, C, 4 * E], F32, kind="ExternalOutput").ap()
    with tile.TileContext(nc) as tc:
        _body(tc, out_ext, ctx_in, q_in, wq_in, wc_in, wm_in)
    nc.compile()
    _NC_CACHE = nc
    return nc


def _run(inputs, trace=False, **kw):
    nc = _build()
    context = np.ascontiguousarray(np.asarray(inputs["context"], dtype=np.float32))
    question = np.ascontiguousarray(np.asarray(inputs["question"], dtype=np.float32))
    wq = np.ascontiguousarray(np.asarray(inputs["w_question"], dtype=np.float32))
    wc = np.ascontiguousarray(np.asarray(inputs["w_context"], dtype=np.float32))
    wm = np.ascontiguousarray(np.asarray(inputs["w_multiple"], dtype=np.float32))
    in_maps = []
    for i in range(NCORES):
        sl = slice(i * BPC, (i + 1) * BPC)
        in_maps.append(
            {
                "context": context[sl],
                "question": question[sl],
                "w_question": wq,
                "w_context": wc,
                "w_multiple": wm,
            }
        )
    res = run_bass_kernel_spmd(
        nc, in_maps, core_ids=list(range(NCORES)), trace=trace, **kw
    )
    out = np.concatenate([res.results[i]["out"] for i in range(NCORES)], axis=0)
    return out, res


def kernel(**inputs):
    try:
        out, _ = _run(inputs, trace=False)
    except Exception:
        # transient device errors (e.g. a wedged core from a prior run)
        # usually clear on retry
        out, _ = _run(inputs, trace=False)
    return out

